# revision 1
# baseline (speedup 1.0000x reference)
"""TRN2 Bass kernel for nn_NodeEmbedding (3-relation GraphConv + PReLU).

Algorithm (per core, nodes 1D-sharded by destination):
  h = PReLU( sum_r (D_in^-1/2 A_r D_out^-1/2 x) W_r + b_r )

Host side (index preprocessing only):
  - global degrees per relation (bincount), edge weights w_e = ns[src]*nd[dst]
  - edges partitioned by destination owner core
  - per (relation, src-chunk) pass: destinations degree-sorted (sigma),
    tiled by 128, padded to a common-across-cores per-tile slot count K_t
  - slot arrays: gather indices (int16, into the src chunk), weights
    (fp32, [128, sum K] layout), scatter positions (int16 natpos)

Device side (all O(E*F)/O(N*F) float work):
  - dma_gather: x rows for each slot, [128, nb, 128] tiles
  - DVE fused multiply-accumulate: acc[p] = sum_k w[p,k] * G[p,k,:]
  - dma_scatter_add: acc rows added into natural-order agg_r in DRAM
  - per 128-node tile: PE transpose of agg_r, PE matmuls (sum_r aggT_r W_r
    + bias via ones-row matmul), PReLU, store y.
"""

import numpy as np
from contextlib import ExitStack

P = 128
F = 128
N_CORES = 8

# problem-size constants (full problem); small-config tests override via Cfg
class Cfg:
    def __init__(self, n_nodes=100000, n_chunks=4, slot_budget=8,
                 group_tiles=8, scat_tiles=4):
        self.GROUP_TILES = group_tiles
        self.SCAT_TILES = scat_tiles
        self.N = n_nodes
        self.SHARD = n_nodes // N_CORES
        self.TILES = (self.SHARD + P - 1) // P
        self.AGG_ROWS = self.TILES * P
        # chunk boundaries (int16 addressable: <= 32767 rows per chunk)
        base = self.N // n_chunks
        assert base <= 32767
        self.CHUNKS = []
        off = 0
        for i in range(n_chunks):
            sz = base if i < n_chunks - 1 else self.N - off
            self.CHUNKS.append((off, sz))
            off += sz
        assert off == self.N
        self.SLOT_BUDGET = slot_budget  # max slot-columns per gather instruction


def _schedule(cfg, inputs):
    """Host preprocessing. Returns (K, cores, norms) where
    K[r][q] = int array [TILES] (common over cores),
    cores[c][r][q] = dict(idx=int16[16,S/16], w=f32[128,S/128], nat=int16[16,NS/16], ntiles)
    """
    N, SHARD, TILES, AGG_ROWS = cfg.N, cfg.SHARD, cfg.TILES, cfg.AGG_ROWS
    NCH = len(cfg.CHUNKS)
    srcs, dsts, ns, nd = [], [], [], []
    for r in range(3):
        src = np.asarray(inputs[f"src{r}"]).astype(np.int64)
        dst = np.asarray(inputs[f"dst{r}"]).astype(np.int64)
        deg_out = np.bincount(src, minlength=N)
        deg_in = np.bincount(dst, minlength=N)
        srcs.append(src)
        dsts.append(dst)
        ns.append((1.0 / np.sqrt(np.maximum(deg_out, 1))).astype(np.float32))
        nd.append((1.0 / np.sqrt(np.maximum(deg_in, 1))).astype(np.float32))

    percore = [[[None] * NCH for _ in range(3)] for _ in range(N_CORES)]
    K = np.zeros((3, NCH, TILES), dtype=np.int64)
    for r in range(3):
        dst_core = dsts[r] // SHARD
        for c in range(N_CORES):
            m = dst_core == c
            s_all, d_all = srcs[r][m], dsts[r][m] - c * SHARD
            for q, (qoff, qsz) in enumerate(cfg.CHUNKS):
                mq = (s_all >= qoff) & (s_all < qoff + qsz)
                s_q, d_q = s_all[mq] - qoff, d_all[mq]
                ld = np.bincount(d_q, minlength=SHARD)
                order = np.argsort(ld, kind="stable")
                ld_sorted = ld[order]
                lds_pad = np.pad(ld_sorted, (0, AGG_ROWS - SHARD))
                K[r, q] = np.maximum(K[r, q], lds_pad.reshape(TILES, P).max(1))
                pos = np.empty(SHARD, np.int64)
                pos[order] = np.arange(SHARD)
                eo = np.argsort(pos[d_q], kind="stable")
                percore[c][r][q] = dict(order=order, s=s_q[eo], d=d_q[eo],
                                        ld_sorted=ld_sorted)

    cores = []
    for c in range(N_CORES):
        rels = []
        for r in range(3):
            chunks = []
            for q, (qoff, qsz) in enumerate(cfg.CHUNKS):
                info = percore[c][r][q]
                Ks = K[r, q]
                live = np.nonzero(Ks > 0)[0]  # tiles with any slots (common!)
                nslots = int(Ks.sum()) * P
                idx_flat = np.zeros(max(nslots, 16), np.int16)
                w_flat = np.zeros(max(nslots, P), np.float32)
                ld_sorted = info["ld_sorted"]
                csum = np.concatenate([[0], np.cumsum(ld_sorted)])
                # coff over LIVE tiles only, in live order
                coff_map = np.full(TILES, -1, np.int64)
                acc_off = 0
                for t in live:
                    coff_map[t] = acc_off
                    acc_off += int(Ks[t]) * P
                ne = len(info["s"])
                if ne:
                    g_ids = np.arange(ne)
                    gpos = np.searchsorted(csum, g_ids, side="right") - 1
                    kk = g_ids - csum[gpos]
                    tt = gpos // P
                    pp = gpos % P
                    i_pos = coff_map[tt] + kk * P + pp
                    assert (coff_map[tt] >= 0).all()
                    idx_flat[i_pos] = info["s"].astype(np.int16)
                    w_flat[i_pos] = (ns[r][info["s"] + qoff]
                                     * nd[r][info["d"] + c * SHARD])
                ar = np.arange(nslots)
                idx_wrap = np.zeros((16, max(nslots // 16, 1)), np.int16)
                if nslots:
                    idx_wrap[ar % 16, ar // 16] = idx_flat[:nslots]
                w_arr = np.zeros((P, max(nslots // P, 1)), np.float32)
                if nslots:
                    w_arr[ar % P, ar // P] = w_flat[:nslots]
                # natpos for live tiles, in live order
                o = info["order"]
                natflat_full = np.empty(AGG_ROWS, np.int64)
                natflat_full[:SHARD] = o
                natflat_full[SHARD:] = np.arange(SHARD, AGG_ROWS)
                nat_list = np.concatenate(
                    [natflat_full[t * P:(t + 1) * P] for t in live]) if len(live) else np.zeros(16, np.int64)
                ns_tot = len(nat_list)
                arn = np.arange(ns_tot)
                nat_wrap = np.zeros((16, max(ns_tot // 16, 1)), np.int16)
                nat_wrap[arn % 16, arn // 16] = nat_list.astype(np.int16)
                # idx/nat are read per-Q7-core from its own 16-partition group:
                # replicate the 16-row wrap across all 128 partitions.
                chunks.append(dict(idx=np.tile(idx_wrap, (8, 1)), w=w_arr,
                                   nat=np.tile(nat_wrap, (8, 1))))
            rels.append(chunks)
        cores.append(rels)
    return K, cores


def _batches(cfg, Ks):
    """Group live tiles into gather batches with sum(K) <= SLOT_BUDGET.
    Returns list of (tile_list, sum_k, slot_col_offset, live_tile_offset)."""
    live = [t for t in range(len(Ks)) if Ks[t] > 0]
    batches = []
    cur, curk = [], 0
    scol0, scat0 = 0, 0
    scol, scat = 0, 0
    for t in live:
        k = int(Ks[t])
        if cur and curk + k > cfg.SLOT_BUDGET:
            batches.append((cur, curk, scol0, scat0))
            scol0, scat0 = scol, scat
            cur, curk = [], 0
        cur.append(t)
        curk += k
        scol += k
        scat += 1
    if cur:
        batches.append((cur, curk, scol0, scat0))
    return batches


def _batch_dims(cfg, K):
    """Max slot-columns and max tiles across all batches (G/scat tile sizes)."""
    gmax, smax = 1, 1
    for r in range(3):
        for q in range(K.shape[1]):
            for tiles_b, nk, _, _ in _batches(cfg, K[r, q]):
                gmax = max(gmax, nk)
                smax = max(smax, len(tiles_b))
    return gmax, smax


def _build_bass(cfg, K):
    """Build the SPMD bass program for the given common K profile."""
    import concourse.bass as bass
    import concourse.bacc as bacc
    import concourse.tile as tile
    import concourse.mybir as mybir
    from concourse.masks import make_identity

    TILES, AGG_ROWS, SHARD = cfg.TILES, cfg.AGG_ROWS, cfg.SHARD
    NCH = len(cfg.CHUNKS)
    dt = mybir.dt

    nc = bacc.Bacc("TRN2", target_bir_lowering=False, debug=False,
                   num_devices=N_CORES)


    xc = [nc.dram_tensor(f"xc{q}", [sz, F], dt.float32, kind="ExternalInput")
          for q, (_, sz) in enumerate(cfg.CHUNKS)]
    idx_t, w_t, nat_t = {}, {}, {}
    for r in range(3):
        for q in range(NCH):
            S = int(K[r, q].sum()) * P
            nlive = int((K[r, q] > 0).sum())
            NS = nlive * P
            idx_t[r, q] = nc.dram_tensor(f"idx{r}_{q}", [P, max(S // 16, 1)],
                                         dt.int16, kind="ExternalInput")
            w_t[r, q] = nc.dram_tensor(f"w{r}_{q}", [P, max(S // P, 1)],
                                       dt.float32, kind="ExternalInput")
            nat_t[r, q] = nc.dram_tensor(f"nat{r}_{q}", [P, max(NS // 16, 1)],
                                         dt.int16, kind="ExternalInput")
    W_t = [nc.dram_tensor(f"W{r}", [F, F], dt.float32, kind="ExternalInput")
           for r in range(3)]
    b_t = [nc.dram_tensor(f"b{r}", [F], dt.float32, kind="ExternalInput")
           for r in range(3)]
    pa_t = nc.dram_tensor("prelu_a", [1], dt.float32, kind="ExternalInput")

    agg_t = [nc.dram_tensor(f"agg{r}", [AGG_ROWS, F], dt.float32,
                            kind="ExternalOutput") for r in range(3)]
    y_t = nc.dram_tensor("y", [SHARD, F], dt.float32, kind="ExternalOutput")

    with tile.TileContext(nc) as tc:
        with ExitStack() as ctx:
            cpool = ctx.enter_context(tc.tile_pool(name="const", bufs=1))
            # constants
            ident = cpool.tile([P, P], dt.float32)
            make_identity(nc, ident[:])
            W_sb = []
            for r in range(3):
                w_ = cpool.tile([F, F], dt.float32, tag=f"W{r}")
                nc.sync.dma_start(w_[:], W_t[r][:, :])
                W_sb.append(w_)
            b_sb = cpool.tile([1, F], dt.float32)
            tmpb = cpool.tile([1, F], dt.float32)
            nc.sync.dma_start(b_sb[:], b_t[0][None, :])
            nc.sync.dma_start(tmpb[:], b_t[1][None, :])
            nc.vector.tensor_add(b_sb[:], b_sb[:], tmpb[:])
            nc.sync.dma_start(tmpb[:], b_t[2][None, :])
            nc.vector.tensor_add(b_sb[:], b_sb[:], tmpb[:])
            ones1 = cpool.tile([1, P], dt.float32)
            nc.vector.memset(ones1[:], 1.0)
            pa_sb = cpool.tile([1, 1], dt.float32)
            nc.sync.dma_start(pa_sb[:], pa_t[None, :])
            am1 = cpool.tile([P, 1], dt.float32)  # (a - 1) broadcast to partitions
            with tc.tile_pool(name="ppsum", bufs=1, space="PSUM") as ppool:
                pa_ps = ppool.tile([P, 1], dt.float32, space="PSUM")
                nc.tensor.matmul(pa_ps[:], lhsT=ones1[:], rhs=pa_sb[:],
                                 start=True, stop=True)
                nc.vector.tensor_scalar_add(am1[:], pa_ps[:], -1.0)

            # ---------------- phase A ----------------
            with ExitStack() as actx:
                ipool = actx.enter_context(tc.tile_pool(name="idx", bufs=2))
                wpool = actx.enter_context(tc.tile_pool(name="w", bufs=2))
                npool = actx.enter_context(tc.tile_pool(name="nat", bufs=2))
                gpool = actx.enter_context(tc.tile_pool(name="g", bufs=3))
                spool = actx.enter_context(tc.tile_pool(name="scat", bufs=3))

                # interleave (q, r) phases; within a phase emit batches
                for q in range(NCH):
                    for r in range(3):
                        Ks = K[r, q]
                        S = int(Ks.sum()) * P
                        if S == 0:
                            continue
                        nlive = int((Ks > 0).sum())
                        idx_sb = ipool.tile([P, S // 16], dt.int16, tag="idx")
                        nc.sync.dma_start(idx_sb[:], idx_t[r, q][:, :S // 16])
                        w_sb = wpool.tile([P, S // P], dt.float32, tag="w")
                        nc.sync.dma_start(w_sb[:], w_t[r, q][:, :S // P])
                        nat_sb = npool.tile([P, nlive * P // 16], dt.int16, tag="nat")
                        nc.sync.dma_start(nat_sb[:], nat_t[r, q][:, :nlive * P // 16])

                        live = [t for t in range(cfg.TILES) if Ks[t] > 0]
                        GB = cfg.SLOT_BUDGET   # slot-cols per gather (<=8: 1024 descs)
                        GRP = cfg.GROUP_TILES  # tiles per scat buffer
                        SC = cfg.SCAT_TILES    # tiles per scatter instr (<=4: 512 rows)
                        scol = 0  # global slot-col counter (idx/w layout order)
                        for g0 in range(0, len(live), GRP):
                            gtiles = live[g0:g0 + GRP]
                            stream = [(j, k) for j, t in enumerate(gtiles)
                                      for k in range(int(Ks[t]))]
                            scat = spool.tile([P, GRP, F], dt.float32, tag="scat")
                            for b0 in range(0, len(stream), GB):
                                chunk = stream[b0:b0 + GB]
                                nb = len(chunk)
                                G = gpool.tile([P, GB, F], dt.float32, tag="G")
                                nc.gpsimd.dma_gather(
                                    out_ap=G[:, :nb, :],
                                    in_ap=xc[q][:, :],
                                    idxs_ap=idx_sb[:, scol * 8:(scol + nb) * 8],
                                    num_idxs=nb * P,
                                    num_idxs_reg=nb * P,
                                    elem_size=F,
                                )
                                for j, (tl, k) in enumerate(chunk):
                                    wc = scol + j
                                    if k == 0:
                                        nc.vector.tensor_scalar_mul(
                                            scat[:, tl, :], G[:, j, :],
                                            w_sb[:, wc:wc + 1])
                                    else:
                                        nc.vector.scalar_tensor_tensor(
                                            out=scat[:, tl, :], in0=G[:, j, :],
                                            scalar=w_sb[:, wc:wc + 1],
                                            in1=scat[:, tl, :],
                                            op0=mybir.AluOpType.mult,
                                            op1=mybir.AluOpType.add)
                                scol += nb
                            for ss in range(0, len(gtiles), SC):
                                nt = min(SC, len(gtiles) - ss)
                                nc.gpsimd.dma_scatter_add(
                                    out_ap=agg_t[r][:, :],
                                    in_ap=scat[:, ss:ss + nt, :],
                                    idxs_ap=nat_sb[:, (g0 + ss) * 8:(g0 + ss + nt) * 8],
                                    num_idxs=nt * P,
                                    num_idxs_reg=nt * P,
                                    elem_size=F,
                                )

            tc.strict_bb_all_engine_barrier()

            # ---------------- phase B ----------------
            with ExitStack() as bctx:
                lpool = bctx.enter_context(tc.tile_pool(name="ld", bufs=3))
                tpool = bctx.enter_context(tc.tile_pool(name="tr", bufs=2))
                ypool = bctx.enter_context(tc.tile_pool(name="y", bufs=3))
                pspool = bctx.enter_context(
                    tc.tile_pool(name="psum", bufs=1, space="PSUM"))
                pypool = bctx.enter_context(
                    tc.tile_pool(name="psumy", bufs=2, space="PSUM"))
                for t in range(TILES):
                    rows = min(P, SHARD - t * P)
                    aggT = []
                    for r in range(3):
                        ag = lpool.tile([P, F], dt.float32, tag=f"ag{r}")
                        nc.sync.dma_start(
                            ag[:], agg_t[r][t * P:(t + 1) * P, :])
                        ps = pspool.tile([P, P], dt.float32, space="PSUM",
                                         tag=f"ps{r}")
                        nc.tensor.transpose(ps[:], ag[:], ident[:])
                        at = tpool.tile([P, P], dt.float32, tag=f"at{r}")
                        nc.scalar.copy(at[:], ps[:])
                        aggT.append(at)
                    py = pypool.tile([P, F], dt.float32, space="PSUM", tag="py")
                    nc.tensor.matmul(py[:], lhsT=ones1[:], rhs=b_sb[:],
                                     start=True, stop=False)
                    for r in range(3):
                        nc.tensor.matmul(py[:], lhsT=aggT[r][:], rhs=W_sb[r][:],
                                         start=False, stop=(r == 2))
                    neg = ypool.tile([P, F], dt.float32, tag="neg")
                    nc.vector.tensor_scalar_min(neg[:], py[:], 0.0)
                    ysb = ypool.tile([P, F], dt.float32, tag="ysb")
                    nc.vector.scalar_tensor_tensor(
                        out=ysb[:], in0=neg[:], scalar=am1[:, :1], in1=py[:],
                        op0=mybir.AluOpType.mult, op1=mybir.AluOpType.add)
                    nc.sync.dma_start(y_t[t * P:t * P + rows, :], ysb[:rows, :])

    nc.compile()
    return nc


_NC_CACHE = {}


def _run(cfg, inputs, trace=False, trace_kwargs=None):
    from concourse.bass_utils import run_bass_kernel_spmd

    x = np.ascontiguousarray(np.asarray(inputs["x"], dtype=np.float32))
    K, cores = _schedule(cfg, inputs)
    key = (cfg.N, len(cfg.CHUNKS), cfg.SLOT_BUDGET, cfg.GROUP_TILES,
           cfg.SCAT_TILES, K.tobytes())
    nc = _NC_CACHE.get(key)
    if nc is None:
        nc = _build_bass(cfg, K)
        _NC_CACHE.clear()
        _NC_CACHE[key] = nc

    in_maps = []
    for c in range(N_CORES):
        m = {}
        for q, (qoff, qsz) in enumerate(cfg.CHUNKS):
            m[f"xc{q}"] = x[qoff:qoff + qsz]
        for r in range(3):
            for q in range(len(cfg.CHUNKS)):
                d = cores[c][r][q]
                m[f"idx{r}_{q}"] = d["idx"]
                m[f"w{r}_{q}"] = d["w"]
                m[f"nat{r}_{q}"] = d["nat"]
            m[f"W{r}"] = np.ascontiguousarray(np.asarray(inputs[f"W{r}"], dtype=np.float32))
            m[f"b{r}"] = np.ascontiguousarray(np.asarray(inputs[f"b{r}"], dtype=np.float32))
        m["prelu_a"] = np.ascontiguousarray(np.asarray(inputs["prelu_a"], dtype=np.float32))
        in_maps.append(m)

    res = run_bass_kernel_spmd(nc, in_maps, core_ids=list(range(N_CORES)),
                               trace=trace, **(trace_kwargs or {}))
    y = np.concatenate([res.results[c]["y"] for c in range(N_CORES)], axis=0)
    return y, res


def kernel(**inputs) -> np.ndarray:
    cfg = Cfg()
    y, _ = _run(cfg, inputs)
    return y.astype(np.float32)


if __name__ == "__main__":
    pass

